# revision 6
# baseline (speedup 1.0000x reference)
"""GRU kernel for Trainium2 (8 NeuronCores, data-parallel over batch).

Problem: T=2048, B=64, F=H=256 flax-style GRU.
  xi = xs @ Wi                       (big input GEMM, precomputed per chunk)
  per step: hg = h @ Wh + bh
            r = sigmoid(xi_r + hg_r); z = sigmoid(xi_z + hg_z)
            n = tanh(xi_n + b_in + r * hg_n)
            h' = (1-z)*n + z*h

Device strategy (per core, local batch BL=8, transposed layout [H,B]):
  - chunks of TC=32 steps; for each chunk the input-projection GEMM
    pre-accumulates (xi + biases) straight into PSUM "staging" banks.
  - scan matmuls accumulate h@Wh on top (start=False), so the sigmoid
    reads its complete argument directly from PSUM.
  - even/odd steps use different PSUM banks so PE writes of step t+1
    never collide with ACT/DVE reads of step t.
  - PSUM banks are step-major interleaved (col = e*32 + sec*8 + b) so all
    hot-path elementwise ops are contiguous 2D APs.
  - h update computed as h' = sig(-u_z)*n + z*h  (one fewer dep hop).
All compute in fp16 on the PE (fp32 PSUM accumulate), elementwise fp32.
"""

import numpy as np

T, B, F, H = 2048, 64, 256, 256
NCORES = 8
BL = B // NCORES  # 8 batch elements per core
TC = 32           # time steps per chunk
EC = TC // 2      # steps per parity (16)
G3 = 3 * H        # 768


def _build_nc(T_=T):
    from contextlib import ExitStack

    import concourse.bass as bass
    import concourse.mybir as mybir
    from concourse import bacc
    from concourse.tile import TileContext

    F16 = mybir.dt.float16
    F32 = mybir.dt.float32
    MULT = mybir.AluOpType.mult
    SIG = mybir.ActivationFunctionType.Sigmoid
    TANH = mybir.ActivationFunctionType.Tanh
    COPY = mybir.ActivationFunctionType.Copy

    nch = T_ // TC
    nc = bacc.Bacc()

    xsT = nc.declare_dram_parameter("xsT", [2, 128, T_ * BL], F16, isOutput=False)
    wi = nc.declare_dram_parameter("wi", [128, 2 * G3], F16, isOutput=False)
    wh = nc.declare_dram_parameter("wh", [128, 2 * G3], F16, isOutput=False)
    brz = nc.declare_dram_parameter("brz", [4, 128], F16, isOutput=False)
    bnn = nc.declare_dram_parameter("bnn", [4, 128], F16, isOutput=False)
    indrz = nc.declare_dram_parameter("indrz", [4, 512], F16, isOutput=False)
    indnn = nc.declare_dram_parameter("indnn", [4, 512], F16, isOutput=False)
    cT0 = nc.declare_dram_parameter("cT", [128, 2 * BL], F16, isOutput=False)
    ysT = nc.declare_dram_parameter("ysT", [2, 128, T_, BL], F16, isOutput=True)

    with TileContext(nc) as tc, ExitStack() as ctx:
        const = ctx.enter_context(tc.tile_pool(name="const", bufs=1))
        bufp = ctx.enter_context(tc.tile_pool(name="bufs", bufs=1))
        step = ctx.enter_context(tc.tile_pool(name="step", bufs=6))
        psum = ctx.enter_context(tc.tile_pool(name="psum", bufs=1, space="PSUM"))

        wi_sb = const.tile([128, 2 * G3], F16, tag="wi")
        wh_sb = const.tile([128, 2 * G3], F16, tag="wh")
        brz_sb = const.tile([4, 128], F16, tag="brz")
        bnn_sb = const.tile([4, 128], F16, tag="bnn")
        indrz_sb = const.tile([4, 512], F16, tag="indrz")
        indnn_sb = const.tile([4, 512], F16, tag="indnn")
        cT_sb = const.tile([128, 2 * BL], F16, tag="cT")
        nc.sync.dma_start(wi_sb[:], wi[:])
        nc.sync.dma_start(wh_sb[:], wh[:])
        nc.sync.dma_start(brz_sb[:], brz[:])
        nc.sync.dma_start(bnn_sb[:], bnn[:])
        nc.sync.dma_start(indrz_sb[:], indrz[:])
        nc.sync.dma_start(indnn_sb[:], indnn[:])
        nc.sync.dma_start(cT_sb[:], cT0[:])

        # PSUM banks: [chunk-buffer][parity].
        # rz bank: col = e*32 + sec*8 + b      (sec in r0,r1,z0,z1)
        # n  bank: col = e*16 + s*8 + b        (s in hn0,hn1), cols 0..255
        #          col = 256 + e*16 + s*8 + b  (s in xn0,xn1 scratch)
        rzb = [[psum.tile([128, 512], F32, tag=f"rz{cb}{p}", name=f"rz{cb}{p}")
                for p in (0, 1)] for cb in (0, 1)]
        nb = [[psum.tile([128, 512], F32, tag=f"nn{cb}{p}", name=f"nn{cb}{p}")
               for p in (0, 1)] for cb in (0, 1)]

        NXS, NYS, NXN = 3, 3, 2
        # xs buf: col = k*256 + t*8 + b (DMA layout)
        # ys buf: col = t*16 + k*8 + b  (time-major: h slices contiguous)
        # xn buf: col = t*16 + s*8 + b
        xs_bufs = [bufp.tile([128, TC * BL * 2], F16, tag=f"xs{i}", name=f"xs{i}")
                   for i in range(NXS)]
        ys_bufs = [bufp.tile([128, TC * BL * 2], F16, tag=f"ys{i}", name=f"ys{i}")
                   for i in range(NYS)]
        xn_bufs = [bufp.tile([128, TC * BL * 2], F32, tag=f"xn{i}", name=f"xn{i}")
                   for i in range(NXN)]

        def dma_xs(c):
            xt = xs_bufs[c % NXS]
            for k in (0, 1):
                nc.sync.dma_start(xt[:, k * 256:(k + 1) * 256],
                                  xsT[k, :, c * TC * BL:(c + 1) * TC * BL])

        def dma_ys(c):
            yt = ys_bufs[c % NYS]
            for k in (0, 1):
                nc.sync.dma_start(
                    ysT[k, :, c * TC:(c + 1) * TC, :],
                    yt[:].rearrange("p (t s b) -> p t s b", s=2, b=BL)[:, :, k, :])

        def xs_parity_ap(c, k, p):
            xt = xs_bufs[c % NXS]
            return xt[:, k * 256:(k + 1) * 256].rearrange(
                "p (e pr b) -> p e pr b", pr=2, b=BL)[:, :, p, :]

        def rz_out_ap(cb, p, sec):
            # GEMM staging target: [128, 16e, 8b] at col e*32 + sec*8
            return rzb[cb][p][:].rearrange(
                "p (e sc b) -> p e sc b", sc=4, b=BL)[:, :, sec, :]

        def nn_out_ap(cb, p, s, scratch):
            off = 2 if scratch else 0
            return nb[cb][p][:].rearrange(
                "p (h e sc b) -> p h e sc b", h=2, sc=2, b=BL)[:, 1 if scratch else 0, :, s, :]

        def prep_ops(c):
            """List of thunks staging chunk c into PSUM buffer c%2."""
            cb = c % 2
            ops = []
            for p in (0, 1):
                ops.append(lambda p=p: nc.tensor.matmul(
                    rzb[cb][p][:], brz_sb[:], indrz_sb[:],
                    start=True, stop=False, skip_group_check=True))
                ops.append(lambda p=p: nc.tensor.matmul(
                    nb[cb][p][:], bnn_sb[:], indnn_sb[:],
                    start=True, stop=False, skip_group_check=True))
            # xi_n into scratch halves, accumulating onto b_in
            for p in (0, 1):
                for s in (0, 1):
                    for k in (0, 1):
                        ops.append(lambda p=p, s=s, k=k: nc.tensor.matmul(
                            nn_out_ap(cb, p, s, True),
                            wi_sb[:, k * G3 + 512 + s * 128:k * G3 + 512 + (s + 1) * 128],
                            xs_parity_ap(c, k, p),
                            start=False, stop=(k == 1), skip_group_check=True))
            # evict xn' = xs@Wi_n + b_in to SBUF (one per parity, 2D psum src)
            for p in (0, 1):
                ops.append(lambda p=p: nc.scalar.activation(
                    xn_bufs[c % NXN][:].rearrange(
                        "p (e pr x) -> p e pr x", pr=2, x=16)[:, :, p, :],
                    nb[cb][p][:, 256:512], COPY))
            # xi_rz into rz staging, accumulating onto bh_rz
            for p in (0, 1):
                for sec in range(4):
                    for k in (0, 1):
                        ops.append(lambda p=p, sec=sec, k=k: nc.tensor.matmul(
                            rz_out_ap(cb, p, sec),
                            wi_sb[:, k * G3 + sec * 128:k * G3 + (sec + 1) * 128],
                            xs_parity_ap(c, k, p),
                            start=False, stop=(k == 1), skip_group_check=True))
            return ops

        def h_ap(c, t, k):
            """[128, 8] AP of k-tile of h entering step (c, t)."""
            if c == 0 and t == 0:
                return cT_sb[:, k * BL:(k + 1) * BL]
            if t == 0:
                return ys_bufs[(c - 1) % NYS][:, (TC - 1) * 16 + k * BL:
                                              (TC - 1) * 16 + (k + 1) * BL]
            return ys_bufs[c % NYS][:, (t - 1) * 16 + k * BL:(t - 1) * 16 + (k + 1) * BL]

        def h_ap2(c, t):
            """[128, 16] AP (both k-tiles) of h entering step (c, t)."""
            if c == 0 and t == 0:
                return cT_sb[:]
            if t == 0:
                return ys_bufs[(c - 1) % NYS][:, (TC - 1) * 16:TC * 16]
            return ys_bufs[c % NYS][:, (t - 1) * 16:t * 16]

        def scan_step(c, t):
            cb = c % 2
            p = t % 2
            e = t // 2
            rbk, nbk = rzb[cb][p], nb[cb][p]
            for sec in range(4):
                for k in (0, 1):
                    nc.tensor.matmul(
                        rbk[:, e * 32 + sec * 8:e * 32 + sec * 8 + 8],
                        wh_sb[:, k * G3 + sec * 128:k * G3 + (sec + 1) * 128],
                        h_ap(c, t, k),
                        start=False, stop=(k == 1), skip_group_check=True)
            for s in (0, 1):
                for k in (0, 1):
                    nc.tensor.matmul(
                        nbk[:, e * 16 + s * 8:e * 16 + s * 8 + 8],
                        wh_sb[:, k * G3 + 512 + s * 128:k * G3 + 512 + (s + 1) * 128],
                        h_ap(c, t, k),
                        start=False, stop=(k == 1), skip_group_check=True)

            rz_t = step.tile([128, 32], F32, tag="rz", name="rz")
            z1_t = step.tile([128, 16], F32, tag="z1", name="z1")
            rn_t = step.tile([128, 16], F32, tag="rn", name="rn")
            pn_t = step.tile([128, 16], F32, tag="pn", name="pn")
            n_t = step.tile([128, 16], F32, tag="n_", name="n_")
            b_t = step.tile([128, 16], F32, tag="b_", name="b_")
            a_t = step.tile([128, 16], F32, tag="a_", name="a_")

            nc.scalar.activation(rz_t[:], rbk[:, e * 32:e * 32 + 32], SIG)
            nc.scalar.activation(z1_t[:], rbk[:, e * 32 + 16:e * 32 + 32],
                                 SIG, scale=-1.0)
            # b = z * h   (off critical path, before n is ready)
            nc.vector.tensor_tensor(b_t[:], rz_t[:, 16:32], h_ap2(c, t), MULT)
            # rn = (hn + bh_n) * r ; n-staging already holds bh_n + h@Wh_n
            nc.vector.tensor_tensor(rn_t[:], nbk[:, e * 16:e * 16 + 16],
                                    rz_t[:, 0:16], MULT)
            nc.vector.tensor_add(pn_t[:], rn_t[:],
                                 xn_bufs[c % NXN][:, t * 16:(t + 1) * 16])
            nc.scalar.activation(n_t[:], pn_t[:], TANH)
            nc.vector.tensor_tensor(a_t[:], z1_t[:], n_t[:], MULT)
            yt = ys_bufs[c % NYS]
            nc.vector.tensor_add(yt[:, t * 16:(t + 1) * 16], a_t[:], b_t[:])

        # ---- prologue
        dma_xs(0)
        for op in prep_ops(0):
            op()
        dma_xs(1)
        # ---- main loop: scan chunk c while staging chunk c+1
        for c in range(nch):
            pending = prep_ops(c + 1) if c + 1 < nch else []
            for t in range(TC):
                scan_step(c, t)
                if t == 0 and c + 2 < nch:
                    dma_xs(c + 2)
                if t < len(pending):
                    pending[t]()
            for op in pending[TC:]:
                op()
            dma_ys(c)

    nc.finalize()
    return nc


def _host_inputs(c, xs, Wi, Wh, bh, b_in, T_=T):
    f16 = np.float16
    wi_h = np.concatenate([Wi[0:128], Wi[128:256]], axis=1).astype(f16)
    wh_h = np.concatenate([Wh[0:128], Wh[128:256]], axis=1).astype(f16)
    brz_h = np.ascontiguousarray(bh[0:512].reshape(4, 128)).astype(f16)
    bnn_h = np.ascontiguousarray(
        np.concatenate([bh[512:768], b_in]).reshape(4, 128)).astype(f16)
    # rz bank: col = e*32 + sec*8 + b  -> section = (col>>3) & 3
    cols = np.arange(512)
    indrz_h = (np.arange(4)[:, None] == ((cols >> 3) & 3)[None, :]).astype(f16)
    # n bank: cols<256: s' = (col>>3)&1 ; cols>=256: s' = 2 + ((col>>3)&1)
    sec_n = np.where(cols < 256, (cols >> 3) & 1, 2 + ((cols >> 3) & 1))
    indnn_h = (np.arange(4)[:, None] == sec_n[None, :]).astype(f16)
    in_maps = []
    for core in range(NCORES):
        xs_c = xs[:, core * BL:(core + 1) * BL, :]
        xsT_h = np.ascontiguousarray(
            np.transpose(xs_c, (2, 0, 1)).reshape(2, 128, T_ * BL)).astype(f16)
        c_c = c[core * BL:(core + 1) * BL]          # [BL, H]
        cT_h = np.ascontiguousarray(
            c_c.T.reshape(2, 128, BL).transpose(1, 0, 2).reshape(128, 2 * BL)
        ).astype(f16)
        in_maps.append({"xsT": xsT_h, "wi": wi_h, "wh": wh_h, "brz": brz_h,
                        "bnn": bnn_h, "indrz": indrz_h, "indnn": indnn_h,
                        "cT": cT_h})
    return in_maps


def _gather(results, T_=T):
    ys = np.empty((T_, B, H), np.float32)
    for core in range(NCORES):
        ysT_c = results[core]["ysT"]  # [2,128,T,BL] f16
        ys[:, core * BL:(core + 1) * BL, :] = (
            np.transpose(ysT_c, (2, 3, 0, 1)).reshape(T_, BL, H).astype(np.float32))
    return ys


def _run(c, xs, Wi, Wh, bh, b_in, T_=T, trace=False):
    from concourse.bass_utils import run_bass_kernel_spmd
    nc = _build_nc(T_)
    in_maps = _host_inputs(c, xs, Wi, Wh, bh, b_in, T_)
    res = run_bass_kernel_spmd(nc, in_maps, list(range(NCORES)), trace=trace)
    ys = _gather(res.results, T_)
    return ys, res


def kernel(c, xs, Wi, Wh, bh, b_in):
    c = np.asarray(c, np.float32)
    xs = np.asarray(xs, np.float32)
    ys, _ = _run(c, xs, np.asarray(Wi, np.float32), np.asarray(Wh, np.float32),
                 np.asarray(bh, np.float32), np.asarray(b_in, np.float32))
    return ys[-1].copy(), ys


# revision 7
# speedup vs baseline: 1.1221x; 1.1221x over previous
"""GRU kernel for Trainium2 (8 NeuronCores, data-parallel over batch).

Problem: T=2048, B=64, F=H=256 flax-style GRU.
  xi = xs @ Wi                       (big input GEMM, precomputed per chunk)
  per step: hg = h @ Wh + bh
            r = sigmoid(xi_r + hg_r); z = sigmoid(xi_z + hg_z)
            n = tanh(xi_n + b_in + r * hg_n)
            h' = (1-z)*n + z*h

Device strategy (per core, local batch BL=8, transposed layout [H,B]):
  - chunks of TC=32 steps; the input-projection GEMM pre-accumulates
    (xi + biases) straight into PSUM staging banks; scan matmuls
    accumulate h@Wh on top (start=False) so sigmoid reads PSUM directly.
  - even/odd steps use different PSUM banks so PE writes of step t+1
    never collide with ACT/DVE reads of step t.
  - n-bank interleaves hn (even cols) with xi_n (odd cols); a single
    tensor_tensor_scan computes r*hn + xi_n in one DVE op.
  - z1 = sigmoid(-u_z) = 1-z via activation scale=-1; update is
    h' = z1*n + (h - z1*h), keeping only sigmoid_r -> scan -> tanh -> mult
    on the critical path.
All compute in fp16 on the PE (fp32 PSUM accumulate), elementwise fp32.
"""

import numpy as np

T, B, F, H = 2048, 64, 256, 256
NCORES = 8
BL = B // NCORES  # 8 batch elements per core
TC = 32           # time steps per chunk
G3 = 3 * H        # 768


def _build_nc(T_=T):
    from contextlib import ExitStack

    import concourse.bass as bass
    import concourse.mybir as mybir
    from concourse import bacc
    from concourse.tile import TileContext

    F16 = mybir.dt.float16
    F32 = mybir.dt.float32
    MULT = mybir.AluOpType.mult
    ADD = mybir.AluOpType.add
    SUB = mybir.AluOpType.subtract
    SIG = mybir.ActivationFunctionType.Sigmoid
    TANH = mybir.ActivationFunctionType.Tanh

    nch = T_ // TC
    nc = bacc.Bacc()

    xsT = nc.declare_dram_parameter("xsT", [2, 128, T_ * BL], F16, isOutput=False)
    wi = nc.declare_dram_parameter("wi", [128, 2 * G3], F16, isOutput=False)
    wh = nc.declare_dram_parameter("wh", [128, 2 * G3], F16, isOutput=False)
    brz = nc.declare_dram_parameter("brz", [4, 128], F16, isOutput=False)
    bnn = nc.declare_dram_parameter("bnn", [4, 128], F16, isOutput=False)
    indrz = nc.declare_dram_parameter("indrz", [4, 512], F16, isOutput=False)
    indnn = nc.declare_dram_parameter("indnn", [4, 512], F16, isOutput=False)
    cT0 = nc.declare_dram_parameter("cT", [128, 2 * BL], F16, isOutput=False)
    ysT = nc.declare_dram_parameter("ysT", [nch, 128, TC * 2 * BL], F16,
                                    isOutput=True)

    with TileContext(nc) as tc, ExitStack() as ctx:
        const = ctx.enter_context(tc.tile_pool(name="const", bufs=1))
        bufp = ctx.enter_context(tc.tile_pool(name="bufs", bufs=1))
        step = ctx.enter_context(tc.tile_pool(name="step", bufs=6))
        psum = ctx.enter_context(tc.tile_pool(name="psum", bufs=1, space="PSUM"))

        wi_sb = const.tile([128, 2 * G3], F16, tag="wi")
        wh_sb = const.tile([128, 2 * G3], F16, tag="wh")
        brz_sb = const.tile([4, 128], F16, tag="brz")
        bnn_sb = const.tile([4, 128], F16, tag="bnn")
        indrz_sb = const.tile([4, 512], F16, tag="indrz")
        indnn_sb = const.tile([4, 512], F16, tag="indnn")
        cT_sb = const.tile([128, 2 * BL], F16, tag="cT")
        nc.sync.dma_start(wi_sb[:], wi[:])
        nc.sync.dma_start(wh_sb[:], wh[:])
        nc.sync.dma_start(brz_sb[:], brz[:])
        nc.sync.dma_start(bnn_sb[:], bnn[:])
        nc.sync.dma_start(indrz_sb[:], indrz[:])
        nc.sync.dma_start(indnn_sb[:], indnn[:])
        nc.sync.dma_start(cT_sb[:], cT0[:])

        # PSUM banks: [chunk-buffer][parity], e = t//2 in 0..15.
        # rz bank: col = e*32 + sec*8 + b        (sec in r0,r1,z0,z1)
        # n  bank: col = e*32 + s*16 + 2*j + par (j = b; even par: hn, odd: xi_n)
        rzb = [[psum.tile([128, 512], F32, tag=f"rz{cb}{p}", name=f"rz{cb}{p}")
                for p in (0, 1)] for cb in (0, 1)]
        nb = [[psum.tile([128, 512], F32, tag=f"nn{cb}{p}", name=f"nn{cb}{p}")
               for p in (0, 1)] for cb in (0, 1)]

        NXS, NYS = 3, 3
        # xs buf: col = k*256 + t*8 + b (DMA layout)
        # ys buf: col = t*16 + k*8 + b  (time-major: h slices contiguous)
        xs_bufs = [bufp.tile([128, TC * BL * 2], F16, tag=f"xs{i}", name=f"xs{i}")
                   for i in range(NXS)]
        ys_bufs = [bufp.tile([128, TC * BL * 2], F16, tag=f"ys{i}", name=f"ys{i}")
                   for i in range(NYS)]
        # interleaved [0 | r] operand for the scan op, one per parity;
        # even cols must stay zero.
        rint = [bufp.tile([128, 32], F32, tag=f"rint{p}", name=f"rint{p}")
                for p in (0, 1)]
        nc.vector.memset(rint[0][:], 0.0)
        nc.vector.memset(rint[1][:], 0.0)

        def dma_xs(c):
            xt = xs_bufs[c % NXS]
            for k in (0, 1):
                nc.sync.dma_start(xt[:, k * 256:(k + 1) * 256],
                                  xsT[k, :, c * TC * BL:(c + 1) * TC * BL])

        def dma_ys(c):
            nc.sync.dma_start(ysT[c, :, :], ys_bufs[c % NYS][:])

        def xs_parity_ap(c, k, p):
            xt = xs_bufs[c % NXS]
            return xt[:, k * 256:(k + 1) * 256].rearrange(
                "p (e pr b) -> p e pr b", pr=2, b=BL)[:, :, p, :]

        def rz_out_ap(cb, p, sec):
            # [128, 16e, 8b] at col e*32 + sec*8
            return rzb[cb][p][:].rearrange(
                "p (e sc b) -> p e sc b", sc=4, b=BL)[:, :, sec, :]

        def nn_gemm_ap(cb, p, s):
            # xi_n staging: [128, 16e, 8j] at col e*32 + s*16 + 2j + 1
            return nb[cb][p][:].rearrange(
                "p (e s j o) -> p e s j o", s=2, j=BL, o=2)[:, :, s, :, 1]

        def prep_ops(c):
            """List of thunks staging chunk c into PSUM buffer c%2."""
            cb = c % 2
            ops = []
            for p in (0, 1):
                ops.append(lambda p=p: nc.tensor.matmul(
                    rzb[cb][p][:], brz_sb[:], indrz_sb[:],
                    start=True, stop=False, skip_group_check=True))
                ops.append(lambda p=p: nc.tensor.matmul(
                    nb[cb][p][:], bnn_sb[:], indnn_sb[:],
                    start=True, stop=False, skip_group_check=True))
            # xi_n into odd cols of n bank, accumulating onto b_in
            for p in (0, 1):
                for s in (0, 1):
                    for k in (0, 1):
                        ops.append(lambda p=p, s=s, k=k: nc.tensor.matmul(
                            nn_gemm_ap(cb, p, s),
                            wi_sb[:, k * G3 + 512 + s * 128:k * G3 + 512 + (s + 1) * 128],
                            xs_parity_ap(c, k, p),
                            start=False, stop=(k == 1), skip_group_check=True))
            # xi_rz into rz staging, accumulating onto bh_rz
            for p in (0, 1):
                for sec in range(4):
                    for k in (0, 1):
                        ops.append(lambda p=p, sec=sec, k=k: nc.tensor.matmul(
                            rz_out_ap(cb, p, sec),
                            wi_sb[:, k * G3 + sec * 128:k * G3 + (sec + 1) * 128],
                            xs_parity_ap(c, k, p),
                            start=False, stop=(k == 1), skip_group_check=True))
            return ops

        def h_ap(c, t, k):
            """[128, 8] AP of k-tile of h entering step (c, t)."""
            if c == 0 and t == 0:
                return cT_sb[:, k * BL:(k + 1) * BL]
            if t == 0:
                return ys_bufs[(c - 1) % NYS][:, (TC - 1) * 16 + k * BL:
                                              (TC - 1) * 16 + (k + 1) * BL]
            return ys_bufs[c % NYS][:, (t - 1) * 16 + k * BL:(t - 1) * 16 + (k + 1) * BL]

        def h_ap2(c, t):
            """[128, 16] AP (both k-tiles) of h entering step (c, t)."""
            if c == 0 and t == 0:
                return cT_sb[:]
            if t == 0:
                return ys_bufs[(c - 1) % NYS][:, (TC - 1) * 16:TC * 16]
            return ys_bufs[c % NYS][:, (t - 1) * 16:t * 16]

        def scan_step(c, t):
            cb = c % 2
            p = t % 2
            e = t // 2
            rbk, nbk = rzb[cb][p], nb[cb][p]
            # h @ Wh accumulation; r sections first (sigmoid_r gates on them),
            # then n, then z.
            for sec in (0, 1):
                for k in (0, 1):
                    nc.tensor.matmul(
                        rbk[:, e * 32 + sec * 8:e * 32 + sec * 8 + 8],
                        wh_sb[:, k * G3 + sec * 128:k * G3 + (sec + 1) * 128],
                        h_ap(c, t, k),
                        start=False, stop=(k == 1), skip_group_check=True)
            for s in (0, 1):
                for k in (0, 1):
                    # hn at even cols e*32 + s*16 + 2j
                    nc.tensor.matmul(
                        nbk[:].rearrange("p (e s j o) -> p e s j o",
                                         s=2, j=BL, o=2)[:, e, s, :, 0],
                        wh_sb[:, k * G3 + 512 + s * 128:k * G3 + 512 + (s + 1) * 128],
                        h_ap(c, t, k),
                        start=False, stop=(k == 1), skip_group_check=True)
            for sec in (2, 3):
                for k in (0, 1):
                    nc.tensor.matmul(
                        rbk[:, e * 32 + sec * 8:e * 32 + sec * 8 + 8],
                        wh_sb[:, k * G3 + sec * 128:k * G3 + (sec + 1) * 128],
                        h_ap(c, t, k),
                        start=False, stop=(k == 1), skip_group_check=True)

            z1_t = step.tile([128, 16], F32, tag="z1", name="z1")
            sc_t = step.tile([128, 32], F32, tag="sc", name="sc")
            n_t = step.tile([128, 16], F32, tag="n_", name="n_")
            q_t = step.tile([128, 16], F32, tag="q_", name="q_")
            b_t = step.tile([128, 16], F32, tag="b_", name="b_")
            a_t = step.tile([128, 16], F32, tag="a_", name="a_")

            # r into odd cols of the interleaved operand
            nc.scalar.activation(
                rint[p][:].rearrange("p (j o) -> p j o", o=2)[:, :, 1],
                rbk[:, e * 32:e * 32 + 16], SIG)
            # z1 = 1 - z
            nc.scalar.activation(z1_t[:], rbk[:, e * 32 + 16:e * 32 + 32],
                                 SIG, scale=-1.0)
            # fused r*hn + xi_n via scan: out odd cols = (r * hn) + xi_n
            nc.vector.tensor_tensor_scan(
                sc_t[:], rint[p][:], nbk[:, e * 32:(e + 1) * 32], 0.0,
                MULT, ADD)
            nc.scalar.activation(
                n_t[:], sc_t[:].rearrange("p (j o) -> p j o", o=2)[:, :, 1],
                TANH)
            # h' = z1*n + (h - z1*h); q and b off the critical path
            nc.vector.tensor_tensor(q_t[:], z1_t[:], h_ap2(c, t), MULT)
            nc.vector.tensor_tensor(b_t[:], h_ap2(c, t), q_t[:], SUB)
            nc.vector.tensor_tensor(a_t[:], z1_t[:], n_t[:], MULT)
            yt = ys_bufs[c % NYS]
            nc.vector.tensor_tensor(yt[:, t * 16:(t + 1) * 16], a_t[:], b_t[:],
                                    ADD)

        # ---- prologue
        dma_xs(0)
        for op in prep_ops(0):
            op()
        dma_xs(1)
        # ---- main loop: scan chunk c while staging chunk c+1
        for c in range(nch):
            pending = prep_ops(c + 1) if c + 1 < nch else []
            for t in range(TC):
                scan_step(c, t)
                if t == 0 and c + 2 < nch:
                    dma_xs(c + 2)
                if t < len(pending):
                    pending[t]()
            for op in pending[TC:]:
                op()
            dma_ys(c)

    nc.finalize()
    return nc


def _host_inputs(c, xs, Wi, Wh, bh, b_in, T_=T):
    f16 = np.float16
    wi_h = np.concatenate([Wi[0:128], Wi[128:256]], axis=1).astype(f16)
    wh_h = np.concatenate([Wh[0:128], Wh[128:256]], axis=1).astype(f16)
    brz_h = np.ascontiguousarray(bh[0:512].reshape(4, 128)).astype(f16)
    bnn_h = np.ascontiguousarray(
        np.concatenate([bh[512:768], b_in]).reshape(4, 128)).astype(f16)
    cols = np.arange(512)
    # rz bank: col = e*32 + sec*8 + b -> section = (col>>3) & 3
    indrz_h = (np.arange(4)[:, None] == ((cols >> 3) & 3)[None, :]).astype(f16)
    # n bank: col = e*32 + s*16 + 2j + par -> s' = (col>>4)&1 + 2*(col&1)
    sec_n = ((cols >> 4) & 1) + 2 * (cols & 1)
    indnn_h = (np.arange(4)[:, None] == sec_n[None, :]).astype(f16)
    in_maps = []
    for core in range(NCORES):
        xs_c = xs[:, core * BL:(core + 1) * BL, :]
        xsT_h = np.ascontiguousarray(
            np.transpose(xs_c, (2, 0, 1)).reshape(2, 128, T_ * BL)).astype(f16)
        c_c = c[core * BL:(core + 1) * BL]          # [BL, H]
        cT_h = np.ascontiguousarray(
            c_c.T.reshape(2, 128, BL).transpose(1, 0, 2).reshape(128, 2 * BL)
        ).astype(f16)
        in_maps.append({"xsT": xsT_h, "wi": wi_h, "wh": wh_h, "brz": brz_h,
                        "bnn": bnn_h, "indrz": indrz_h, "indnn": indnn_h,
                        "cT": cT_h})
    return in_maps


def _gather(results, T_=T):
    nch = T_ // TC
    ys = np.empty((T_, B, H), np.float32)
    for core in range(NCORES):
        arr = results[core]["ysT"]  # [nch, 128, 512] f16
        # col = tt*16 + k*8 + b  ->  ys[c*32+tt, b, k*128+p]
        a5 = arr.reshape(nch, 128, TC, 2, BL)
        ys[:, core * BL:(core + 1) * BL, :] = (
            a5.transpose(0, 2, 4, 3, 1).reshape(T_, BL, H).astype(np.float32))
    return ys


def _run(c, xs, Wi, Wh, bh, b_in, T_=T, trace=False):
    from concourse.bass_utils import run_bass_kernel_spmd
    nc = _build_nc(T_)
    in_maps = _host_inputs(c, xs, Wi, Wh, bh, b_in, T_)
    res = run_bass_kernel_spmd(nc, in_maps, list(range(NCORES)), trace=trace)
    ys = _gather(res.results, T_)
    return ys, res


def kernel(c, xs, Wi, Wh, bh, b_in):
    c = np.asarray(c, np.float32)
    xs = np.asarray(xs, np.float32)
    ys, _ = _run(c, xs, np.asarray(Wi, np.float32), np.asarray(Wh, np.float32),
                 np.asarray(bh, np.float32), np.asarray(b_in, np.float32))
    return ys[-1].copy(), ys


# revision 9
# speedup vs baseline: 1.2231x; 1.0899x over previous
"""GRU kernel for Trainium2 (8 NeuronCores, data-parallel over batch).

Problem: T=2048, B=64, F=H=256 flax-style GRU.
  xi = xs @ Wi                       (big input GEMM, precomputed per chunk)
  per step: hg = h @ Wh + bh
            r = sigmoid(xi_r + hg_r); z = sigmoid(xi_z + hg_z)
            n = tanh(xi_n + b_in + r * hg_n)
            h' = (1-z)*n + z*h

Device strategy (per core, local batch BL=8, transposed layout [H,B]):
  - chunks of TC=32 steps; the input-projection GEMM pre-accumulates
    (xi + biases) straight into PSUM staging banks; scan matmuls
    accumulate h@Wh on top (start=False) so sigmoid reads PSUM directly.
  - even/odd steps use different PSUM banks so PE writes of step t+1
    never collide with ACT/DVE reads of step t.
  - n-bank interleaves hn (even cols) with xi_n (odd cols); a single
    tensor_tensor_scan computes r*hn + xi_n in one DVE op.
  - z1 = sigmoid(-u_z) = 1-z via activation scale=-1; update is
    h' = z1*n + (h - z1*h), keeping only sigmoid_r -> scan -> tanh -> mult
    on the critical path.
All compute in fp16 on the PE (fp32 PSUM accumulate), elementwise fp32.
"""

import numpy as np

T, B, F, H = 2048, 64, 256, 256
NCORES = 8
BL = B // NCORES  # 8 batch elements per core
TC = 32           # time steps per chunk
G3 = 3 * H        # 768


def _build_nc(T_=T):
    from contextlib import ExitStack

    import concourse.bass as bass
    import concourse.mybir as mybir
    from concourse import bacc
    from concourse.tile import TileContext

    F16 = mybir.dt.float16
    F32 = mybir.dt.float32
    MULT = mybir.AluOpType.mult
    ADD = mybir.AluOpType.add
    SUB = mybir.AluOpType.subtract
    SIG = mybir.ActivationFunctionType.Sigmoid
    TANH = mybir.ActivationFunctionType.Tanh

    nch = T_ // TC
    nc = bacc.Bacc()

    xsT = nc.declare_dram_parameter("xsT", [2, 128, T_ * BL], F16, isOutput=False)
    wi = nc.declare_dram_parameter("wi", [128, 2 * G3], F16, isOutput=False)
    wh = nc.declare_dram_parameter("wh", [128, 2 * G3], F16, isOutput=False)
    brz = nc.declare_dram_parameter("brz", [4, 128], F16, isOutput=False)
    bnn = nc.declare_dram_parameter("bnn", [4, 128], F16, isOutput=False)
    indx = nc.declare_dram_parameter("indx", [4, 512], F16, isOutput=False)
    indy = nc.declare_dram_parameter("indy", [4, 512], F16, isOutput=False)
    indnn = nc.declare_dram_parameter("indnn", [4, 512], F16, isOutput=False)
    cT0 = nc.declare_dram_parameter("cT", [128, 2 * BL], F16, isOutput=False)
    ysT = nc.declare_dram_parameter("ysT", [nch, 128, TC * 2 * BL], F16,
                                    isOutput=True)

    with TileContext(nc) as tc, ExitStack() as ctx:
        const = ctx.enter_context(tc.tile_pool(name="const", bufs=1))
        bufp = ctx.enter_context(tc.tile_pool(name="bufs", bufs=1))
        step = ctx.enter_context(tc.tile_pool(name="step", bufs=6))
        psum = ctx.enter_context(tc.tile_pool(name="psum", bufs=1, space="PSUM"))

        wi_sb = const.tile([128, 2 * G3], F16, tag="wi")
        wh_sb = const.tile([128, 2 * G3], F16, tag="wh")
        brz_sb = const.tile([4, 128], F16, tag="brz")
        bnn_sb = const.tile([4, 128], F16, tag="bnn")
        indx_sb = const.tile([4, 512], F16, tag="indx")
        indy_sb = const.tile([4, 512], F16, tag="indy")
        indnn_sb = const.tile([4, 512], F16, tag="indnn")
        cT_sb = const.tile([128, 2 * BL], F16, tag="cT")
        nc.sync.dma_start(wi_sb[:], wi[:])
        nc.sync.dma_start(wh_sb[:], wh[:])
        nc.sync.dma_start(brz_sb[:], brz[:])
        nc.sync.dma_start(bnn_sb[:], bnn[:])
        nc.sync.dma_start(indx_sb[:], indx[:])
        nc.sync.dma_start(indy_sb[:], indy[:])
        nc.sync.dma_start(indnn_sb[:], indnn[:])
        nc.sync.dma_start(cT_sb[:], cT0[:])

        # PSUM banks: [chunk-buffer], e = t//2 in 0..15.
        # X bank: [r-par0 (cols 0..255) | z-par1 (256..511)], col = base + e*16 + sec*8 + b
        # Y bank: [z-par0 | r-par1]
        # n bank (per parity): col = e*32 + s*16 + 2*j + par(0=hn,1=xi_n)
        xb = [psum.tile([128, 512], F32, tag=f"xb{cb}", name=f"xb{cb}")
              for cb in (0, 1)]
        yb = [psum.tile([128, 512], F32, tag=f"yb{cb}", name=f"yb{cb}")
              for cb in (0, 1)]
        nb = [[psum.tile([128, 512], F32, tag=f"nn{cb}{p}", name=f"nn{cb}{p}")
               for p in (0, 1)] for cb in (0, 1)]

        def r_bank(cb, p):
            return (xb[cb], 0) if p == 0 else (yb[cb], 256)

        def z_bank(cb, p):
            return (yb[cb], 0) if p == 0 else (xb[cb], 256)

        NXS, NYS = 3, 3
        # xs buf: col = k*256 + t*8 + b (DMA layout)
        # ys buf: col = t*16 + k*8 + b  (time-major: h slices contiguous)
        xs_bufs = [bufp.tile([128, TC * BL * 2], F16, tag=f"xs{i}", name=f"xs{i}")
                   for i in range(NXS)]
        ys_bufs = [bufp.tile([128, TC * BL * 2], F16, tag=f"ys{i}", name=f"ys{i}")
                   for i in range(NYS)]
        # interleaved [0 | r] operand for the scan op, one per parity;
        # even cols must stay zero.
        rint = [bufp.tile([128, 32], F32, tag=f"rint{p}", name=f"rint{p}")
                for p in (0, 1)]
        nc.vector.memset(rint[0][:], 0.0)
        nc.vector.memset(rint[1][:], 0.0)

        def dma_xs(c):
            xt = xs_bufs[c % NXS]
            for k in (0, 1):
                nc.sync.dma_start(xt[:, k * 256:(k + 1) * 256],
                                  xsT[k, :, c * TC * BL:(c + 1) * TC * BL])

        def dma_ys(c):
            nc.sync.dma_start(ysT[c, :, :], ys_bufs[c % NYS][:])

        def xs_parity_ap(c, k, p):
            xt = xs_bufs[c % NXS]
            return xt[:, k * 256:(k + 1) * 256].rearrange(
                "p (e pr b) -> p e pr b", pr=2, b=BL)[:, :, p, :]

        def r_gemm_ap(cb, p, sec):
            bank, base = r_bank(cb, p)
            return bank[:, base:base + 256].rearrange(
                "p (e sc b) -> p e sc b", sc=2, b=BL)[:, :, sec, :]

        def z_gemm_ap(cb, p, sec):
            bank, base = z_bank(cb, p)
            return bank[:, base:base + 256].rearrange(
                "p (e sc b) -> p e sc b", sc=2, b=BL)[:, :, sec, :]

        def nn_gemm_ap(cb, p, s):
            # xi_n staging: [128, 16e, 8j] at col e*32 + s*16 + 2j + 1
            return nb[cb][p][:].rearrange(
                "p (e s j o) -> p e s j o", s=2, j=BL, o=2)[:, :, s, :, 1]

        def prep_ops(c):
            """List of thunks staging chunk c into PSUM buffer c%2."""
            cb = c % 2
            ops = []
            ops.append(lambda: nc.tensor.matmul(
                xb[cb][:], brz_sb[:], indx_sb[:],
                start=True, stop=False, skip_group_check=True))
            ops.append(lambda: nc.tensor.matmul(
                yb[cb][:], brz_sb[:], indy_sb[:],
                start=True, stop=False, skip_group_check=True))
            for p in (0, 1):
                ops.append(lambda p=p: nc.tensor.matmul(
                    nb[cb][p][:], bnn_sb[:], indnn_sb[:],
                    start=True, stop=False, skip_group_check=True))
            # xi_n into odd cols of n bank, accumulating onto b_in
            for p in (0, 1):
                for s in (0, 1):
                    for k in (0, 1):
                        ops.append(lambda p=p, s=s, k=k: nc.tensor.matmul(
                            nn_gemm_ap(cb, p, s),
                            wi_sb[:, k * G3 + 512 + s * 128:k * G3 + 512 + (s + 1) * 128],
                            xs_parity_ap(c, k, p),
                            start=False, stop=(k == 1), skip_group_check=True))
            # xi_r / xi_z staging, accumulating onto bh
            for p in (0, 1):
                for sec in (0, 1):
                    for k in (0, 1):
                        ops.append(lambda p=p, sec=sec, k=k: nc.tensor.matmul(
                            r_gemm_ap(cb, p, sec),
                            wi_sb[:, k * G3 + sec * 128:k * G3 + (sec + 1) * 128],
                            xs_parity_ap(c, k, p),
                            start=False, stop=(k == 1), skip_group_check=True))
                for sec in (0, 1):
                    for k in (0, 1):
                        ops.append(lambda p=p, sec=sec, k=k: nc.tensor.matmul(
                            z_gemm_ap(cb, p, sec),
                            wi_sb[:, k * G3 + 256 + sec * 128:k * G3 + 256 + (sec + 1) * 128],
                            xs_parity_ap(c, k, p),
                            start=False, stop=(k == 1), skip_group_check=True))
            return ops

        def h_ap(c, t, k):
            """[128, 8] AP of k-tile of h entering step (c, t)."""
            if c == 0 and t == 0:
                return cT_sb[:, k * BL:(k + 1) * BL]
            if t == 0:
                return ys_bufs[(c - 1) % NYS][:, (TC - 1) * 16 + k * BL:
                                              (TC - 1) * 16 + (k + 1) * BL]
            return ys_bufs[c % NYS][:, (t - 1) * 16 + k * BL:(t - 1) * 16 + (k + 1) * BL]

        def h_ap2(c, t):
            """[128, 16] AP (both k-tiles) of h entering step (c, t)."""
            if c == 0 and t == 0:
                return cT_sb[:]
            if t == 0:
                return ys_bufs[(c - 1) % NYS][:, (TC - 1) * 16:TC * 16]
            return ys_bufs[c % NYS][:, (t - 1) * 16:t * 16]

        # a/b tiles of the previous step feed the next step's matmuls:
        # hg_{t+1} = a_t @ Wh + b_t @ Wh  (h_{t+1} = a_t + b_t by linearity),
        # so PE can start as soon as a_t lands; h' is off the critical path.
        ab_prev = [None]

        def burst(cb, p, e, rhs16, order):
            """One 12-matmul pass accumulating rhs16 @ Wh into step slots."""
            rbk, rbase = r_bank(cb, p)
            zbk, zbase = z_bank(cb, p)
            nbk = nb[cb][p]
            for part in order:
                if part == "r":
                    for sec in (0, 1):
                        for k in (0, 1):
                            nc.tensor.matmul(
                                rbk[:, rbase + e * 16 + sec * 8:
                                    rbase + e * 16 + sec * 8 + 8],
                                wh_sb[:, k * G3 + sec * 128:k * G3 + (sec + 1) * 128],
                                rhs16[:, k * BL:(k + 1) * BL],
                                start=False, stop=(k == 1), skip_group_check=True)
                elif part == "n":
                    for s in (0, 1):
                        for k in (0, 1):
                            nc.tensor.matmul(
                                nbk[:].rearrange("p (e s j o) -> p e s j o",
                                                 s=2, j=BL, o=2)[:, e, s, :, 0],
                                wh_sb[:, k * G3 + 512 + s * 128:k * G3 + 512 + (s + 1) * 128],
                                rhs16[:, k * BL:(k + 1) * BL],
                                start=False, stop=(k == 1), skip_group_check=True)
                else:
                    for sec in (0, 1):
                        for k in (0, 1):
                            nc.tensor.matmul(
                                zbk[:, zbase + e * 16 + sec * 8:
                                    zbase + e * 16 + sec * 8 + 8],
                                wh_sb[:, k * G3 + 256 + sec * 128:k * G3 + 256 + (sec + 1) * 128],
                                rhs16[:, k * BL:(k + 1) * BL],
                                start=False, stop=(k == 1), skip_group_check=True)

        def scan_step(c, t):
            cb = c % 2
            p = t % 2
            e = t // 2
            rbk, rbase = r_bank(cb, p)
            zbk, zbase = z_bank(cb, p)
            nbk = nb[cb][p]
            if c == 0 and t == 0:
                burst(cb, p, e, cT_sb[:], ("r", "n", "z"))
            else:
                a_prev, b_prev = ab_prev[0]
                burst(cb, p, e, b_prev[:], ("r", "n", "z"))
                burst(cb, p, e, a_prev[:], ("r", "n", "z"))

            z1_t = step.tile([128, 16], F32, tag="z1", name="z1")
            sc_t = step.tile([128, 32], F32, tag="sc", name="sc")
            n_t = step.tile([128, 16], F32, tag="n_", name="n_")
            q_t = step.tile([128, 16], F32, tag="q_", name="q_")
            b_t = step.tile([128, 16], F16, tag="b_", name="b_")
            a_t = step.tile([128, 16], F16, tag="a_", name="a_")

            # r into odd cols of the interleaved operand
            nc.scalar.activation(
                rint[p][:].rearrange("p (j o) -> p j o", o=2)[:, :, 1],
                rbk[:, rbase + e * 16:rbase + e * 16 + 16], SIG)
            # z1 = 1 - z
            nc.scalar.activation(z1_t[:],
                                 zbk[:, zbase + e * 16:zbase + e * 16 + 16],
                                 SIG, scale=-1.0)
            # fused r*hn + xi_n via scan: out odd cols = (r * hn) + xi_n
            nc.vector.tensor_tensor_scan(
                sc_t[:], rint[p][:], nbk[:, e * 32:(e + 1) * 32], 0.0,
                MULT, ADD)
            nc.scalar.activation(
                n_t[:], sc_t[:].rearrange("p (j o) -> p j o", o=2)[:, :, 1],
                TANH)
            # h' = z1*n + (h - z1*h); q, b, h' off the critical path
            nc.vector.tensor_tensor(q_t[:], z1_t[:], h_ap2(c, t), MULT)
            nc.vector.tensor_tensor(b_t[:], h_ap2(c, t), q_t[:], SUB)
            nc.vector.tensor_tensor(a_t[:], z1_t[:], n_t[:], MULT)
            yt = ys_bufs[c % NYS]
            nc.vector.tensor_tensor(yt[:, t * 16:(t + 1) * 16], a_t[:], b_t[:],
                                    ADD)
            ab_prev[0] = (a_t, b_t)

        # ---- prologue
        dma_xs(0)
        for op in prep_ops(0):
            op()
        dma_xs(1)
        # ---- main loop: scan chunk c while staging chunk c+1
        for c in range(nch):
            pending = prep_ops(c + 1) if c + 1 < nch else []
            for t in range(TC):
                scan_step(c, t)
                if t == 0 and c + 2 < nch:
                    dma_xs(c + 2)
                if t < len(pending):
                    pending[t]()
            for op in pending[TC:]:
                op()
            dma_ys(c)

    nc.finalize()
    return nc


def _host_inputs(c, xs, Wi, Wh, bh, b_in, T_=T):
    f16 = np.float16
    wi_h = np.concatenate([Wi[0:128], Wi[128:256]], axis=1).astype(f16)
    wh_h = np.concatenate([Wh[0:128], Wh[128:256]], axis=1).astype(f16)
    brz_h = np.ascontiguousarray(bh[0:512].reshape(4, 128)).astype(f16)
    bnn_h = np.ascontiguousarray(
        np.concatenate([bh[512:768], b_in]).reshape(4, 128)).astype(f16)
    cols = np.arange(512)
    # X bank: [r|z] halves, sec-in-half = (col>>3)&1; brz rows = [r0,r1,z0,z1]
    sec_x = np.where(cols < 256, (cols >> 3) & 1, 2 + ((cols >> 3) & 1))
    indx_h = (np.arange(4)[:, None] == sec_x[None, :]).astype(f16)
    sec_y = np.where(cols < 256, 2 + ((cols >> 3) & 1), (cols >> 3) & 1)
    indy_h = (np.arange(4)[:, None] == sec_y[None, :]).astype(f16)
    # n bank: col = e*32 + s*16 + 2j + par -> s' = (col>>4)&1 + 2*(col&1)
    sec_n = ((cols >> 4) & 1) + 2 * (cols & 1)
    indnn_h = (np.arange(4)[:, None] == sec_n[None, :]).astype(f16)
    in_maps = []
    for core in range(NCORES):
        xs_c = xs[:, core * BL:(core + 1) * BL, :]
        xsT_h = np.ascontiguousarray(
            np.transpose(xs_c, (2, 0, 1)).reshape(2, 128, T_ * BL)).astype(f16)
        c_c = c[core * BL:(core + 1) * BL]          # [BL, H]
        cT_h = np.ascontiguousarray(
            c_c.T.reshape(2, 128, BL).transpose(1, 0, 2).reshape(128, 2 * BL)
        ).astype(f16)
        in_maps.append({"xsT": xsT_h, "wi": wi_h, "wh": wh_h, "brz": brz_h,
                        "bnn": bnn_h, "indx": indx_h, "indy": indy_h, "indnn": indnn_h,
                        "cT": cT_h})
    return in_maps


def _gather(results, T_=T):
    nch = T_ // TC
    ys = np.empty((T_, B, H), np.float32)
    for core in range(NCORES):
        arr = results[core]["ysT"]  # [nch, 128, 512] f16
        # col = tt*16 + k*8 + b  ->  ys[c*32+tt, b, k*128+p]
        a5 = arr.reshape(nch, 128, TC, 2, BL)
        ys[:, core * BL:(core + 1) * BL, :] = (
            a5.transpose(0, 2, 4, 3, 1).reshape(T_, BL, H).astype(np.float32))
    return ys


def _run(c, xs, Wi, Wh, bh, b_in, T_=T, trace=False):
    from concourse.bass_utils import run_bass_kernel_spmd
    nc = _build_nc(T_)
    in_maps = _host_inputs(c, xs, Wi, Wh, bh, b_in, T_)
    res = run_bass_kernel_spmd(nc, in_maps, list(range(NCORES)), trace=trace)
    ys = _gather(res.results, T_)
    return ys, res


def kernel(c, xs, Wi, Wh, bh, b_in):
    c = np.asarray(c, np.float32)
    xs = np.asarray(xs, np.float32)
    ys, _ = _run(c, xs, np.asarray(Wi, np.float32), np.asarray(Wh, np.float32),
                 np.asarray(bh, np.float32), np.asarray(b_in, np.float32))
    return ys[-1].copy(), ys
